# revision 8
# baseline (speedup 1.0000x reference)
"""Grouped-GEMM MoE kernel for Trainium2, expert-parallel across 8 NeuronCores.

Strategy (matches the module's expert-parallel path):
  - Host routes: sort the T*topk routed pairs by expert id; core e gets the
    tokens routed to expert e (padded to a common capacity C).
  - Device (per core): Y = gelu(X @ W1 + b1) @ W2 + b2, scaled per-row by the
    routing weight. Two chained GEMMs on the PE array in bf16 with fp32 PSUM
    accumulation; gelu fused into the PSUM->SBUF eviction on the ACT engine.
  - Host combines: scatter rows back by pair index and sum the topk=2 slots.

Perf notes (trace-driven):
  - W1 is loaded as 16 independent 256 KB chunks (one per output m-tile) so
    the first matmul starts ~4us into the kernel instead of waiting ~34us for
    one monolithic 4 MB DMA.
  - Capacity is padded to a multiple of 128 (not 512): full 512-token tiles
    plus one remainder tile, cutting ~8% of padded matmul work.
  - Outputs are stored as bf16, halving the output DMA traffic.

Problem shapes (hardcoded per contract): B=4, S=4096, H=1024, F=2048, E=8,
TOPK=2.
"""

import sys

for _p in ("/opt/trn_rl_repo", "/opt/pypackages"):
    if _p not in sys.path:
        sys.path.insert(0, _p)

import ml_dtypes
import numpy as np

import concourse.bass as bass  # noqa: F401  (engine types come via bacc)
import concourse.mybir as mybir
import concourse.tile as tile
from concourse import bacc
from concourse.bass_utils import run_bass_kernel_spmd

H = 1024
F = 2048
E = 8
TOPK = 2
N_CORES = 8
P = 128
NTILE = 512          # full token-tile width (matmul moving dim)
KK1 = H // P         # 8  k-steps in GEMM1
KK2 = F // P         # 16 k-steps in GEMM2
MT1 = F // P         # 16 output m-tiles in GEMM1
NT2 = H // NTILE     # 2  output n-tiles in GEMM2
W2G = 4              # w2 DMA split: 4 chunks of 4 k-slices

BF16 = mybir.dt.bfloat16
F32 = mybir.dt.float32

_CACHE = {}
last_result = None   # BassKernelResults of the most recent device run


def _build(C):
    """Build + compile the per-core program for capacity C (multiple of 128)."""
    assert C % P == 0
    nfull = C // NTILE            # full 512-token tiles
    rem = C - nfull * NTILE       # remainder tile width (0/128/256/384)
    widths = [NTILE] * nfull + ([rem] if rem else [])
    nmc = C // P                  # token m-chunks (GEMM2 output rows)

    # num_devices=1: the per-core programs are fully independent (no
    # collectives), so skip the cross-core NEFF entry/exit barriers.
    nc = bacc.Bacc("TRN2", target_bir_lowering=False, debug=False,
                   num_devices=1)

    # DRAM I/O. Layouts are chosen so every DMA is a plain strided copy:
    #   xt[p, kk, c]       = X[c, kk*128+p]   (tokens transposed, H in 8x128)
    #   w1[p, m, kk*128+j] = W1[kk*128+p, m*128+j]  (per-m contiguous chunks)
    #   w2[p, kk, j]       = W2[kk*128+p, j]
    #   b1[p, m]           = b1[m*128+p]
    #   b2r[p, j]          = b2[j]            (pre-replicated across partitions)
    #   rw[p, mc]          = rweight[mc*128+p]
    #   y[mc, p, j]        = Y[mc*128+p, j]   (bf16)
    xt_d = nc.dram_tensor("xt", [P, KK1, C], BF16, kind="ExternalInput")
    w1_d = nc.dram_tensor("w1", [P, MT1, KK1 * P], BF16, kind="ExternalInput")
    w2_d = nc.dram_tensor("w2", [P, KK2, H], BF16, kind="ExternalInput")
    b1_d = nc.dram_tensor("b1", [P, MT1], F32, kind="ExternalInput")
    rw_d = nc.dram_tensor("rw", [P, nmc], F32, kind="ExternalInput")
    y_d = nc.dram_tensor("y", [nmc, P, H], BF16, kind="ExternalOutput")

    with tile.TileContext(nc) as tc:
        with (
            tc.tile_pool(name="const", bufs=1) as const,
            tc.tile_pool(name="xin", bufs=3) as xin,
            tc.tile_pool(name="gact", bufs=3) as gact,
            tc.tile_pool(name="yout", bufs=4) as yout,
            tc.tile_pool(name="psg", bufs=4, space="PSUM") as psg,
            tc.tile_pool(name="psy", bufs=3, space="PSUM") as psy,
        ):
            # First x tile in 4 chunks (2 k-slices each) so GEMM1 m=0 can
            # start as soon as the first 256 KB lands. DMA issue on the Sync
            # engine serializes at ~0.6us per dma_start, so the front only
            # carries the chunks that actually gate compute; the bulk of the
            # weights goes out as a few large transfers.
            xt0 = xin.tile([P, KK1, NTILE if widths else P], BF16, tag="xt")
            for q in range(4):
                eng = nc.sync if q < 2 else nc.gpsimd
                eng.dma_start(xt0[:, 2 * q:2 * q + 2, :],
                              xt_d[:, 2 * q:2 * q + 2, :widths[0]])

            # W1 m-tiles 0-3 as independent 256 KB chunks issued from the
            # Scalar engine (its HWDGE queues are separate from Sync's, so
            # these land in parallel with the xt0 chunks).
            w1_t = []
            for m in range(4):
                t = const.tile([P, KK1 * P], BF16, tag=f"w1_{m}")
                nc.scalar.dma_start(t[:], w1_d[:, m, :])
                w1_t.append(t)
            b1_sb = const.tile([P, MT1], F32)
            nc.scalar.dma_start(b1_sb[:], b1_d[:])

            # Rest of W1 in two 1.5 MB transfers (needed ~20us+ in).
            w1g = []
            for g in range(2):
                t = const.tile([P, 6, KK1 * P], BF16, tag=f"w1g_{g}")
                nc.sync.dma_start(t[:], w1_d[:, 4 + 6 * g:10 + 6 * g, :])
                w1g.append(t)

            def w1s(m, kk):
                if m < 4:
                    return w1_t[m][:, kk * P:(kk + 1) * P]
                g, mi = divmod(m - 4, 6)
                return w1g[g][:, mi, kk * P:(kk + 1) * P]

            # W2 as one 4 MB transfer (needed only when GEMM2 starts ~50us).
            w2_sb = const.tile([P, KK2, H], BF16)
            nc.sync.dma_start(w2_sb[:], w2_d[:])

            rw_sb = const.tile([P, nmc], F32)
            nc.sync.dma_start(rw_sb[:], rw_d[:])

            cbase = 0
            for ct, w in enumerate(widths):
                if ct == 0:
                    xt_sb = xt0
                else:
                    xt_sb = xin.tile([P, KK1, w], BF16, tag="xt")
                    nc.sync.dma_start(
                        xt_sb[:], xt_d[:, :, cbase:cbase + w])

                # GEMM1: GT[f, c] = sum_h W1[h, f] * XT[h, c], then
                # gelu(.+b1) on eviction. F on partitions, tokens on free.
                gt_sb = gact.tile([P, MT1, w], BF16, tag="gt")
                for m in range(MT1):
                    pg = psg.tile([P, w], F32, tag="pg")
                    for kk in range(KK1):
                        nc.tensor.matmul(
                            pg[:],
                            w1s(m, kk),
                            xt_sb[:, kk, :w],
                            start=(kk == 0), stop=(kk == KK1 - 1))
                    nc.scalar.activation(
                        gt_sb[:, m, :], pg[:],
                        mybir.ActivationFunctionType.Gelu,
                        bias=b1_sb[:, m:m + 1])

                # GEMM2: Y[c, j] = sum_f GT[f, c] * W2[f, j]; tokens on
                # partitions. Evict: (+b2) * routing_weight -> bf16, store.
                for mo in range(w // P):
                    mc = cbase // P + mo
                    for n in range(NT2):
                        py = psy.tile([P, NTILE], F32, tag="py")
                        for kk in range(KK2):
                            nc.tensor.matmul(
                                py[:],
                                gt_sb[:, kk, mo * P:(mo + 1) * P],
                                w2_sb[:, kk, n * NTILE:(n + 1) * NTILE],
                                start=(kk == 0), stop=(kk == KK2 - 1))
                        yo = yout.tile([P, NTILE], BF16, tag="yo")
                        nc.vector.tensor_scalar_mul(
                            yo[:], py[:], rw_sb[:, mc:mc + 1])
                        nc.sync.dma_start(
                            y_d[mc, :, n * NTILE:(n + 1) * NTILE], yo[:])
                cbase += w

    nc.compile()
    return nc


def kernel(hidden_states, expert_weights, top_experts, w1, b1, w2, b2,
           _trace=False):
    global last_result
    x = np.asarray(hidden_states, dtype=np.float32)
    fw = np.asarray(expert_weights, dtype=np.float32).reshape(-1)
    fe = np.asarray(top_experts).reshape(-1).astype(np.int64)
    w1 = np.asarray(w1, dtype=np.float32)
    b1 = np.asarray(b1, dtype=np.float32)
    w2 = np.asarray(w2, dtype=np.float32)
    b2 = np.asarray(b2, dtype=np.float32)

    b, s, h = x.shape
    T = b * s
    xf = x.reshape(T, h)
    npair = T * TOPK

    # Host-side routing: stable sort of pair indices by expert id.
    order = np.argsort(fe, kind="stable")
    counts = np.bincount(fe, minlength=E)
    starts = np.concatenate([[0], np.cumsum(counts)])
    C = max(int(-(-counts.max() // P)) * P, P)
    nmc = C // P

    key = C
    if key not in _CACHE:
        _CACHE[key] = _build(C)
    nc = _CACHE[key]

    in_maps = []
    for e in range(E):
        idx = order[starts[e]:starts[e + 1]]
        cnt = len(idx)
        xe = np.zeros((C, H), np.float32)
        xe[:cnt] = xf[idx // TOPK]
        xt = np.ascontiguousarray(
            xe.T.reshape(KK1, P, C).transpose(1, 0, 2)).astype(
                ml_dtypes.bfloat16)
        rwe = np.zeros(C, np.float32)
        rwe[:cnt] = fw[idx]
        in_maps.append({
            "xt": xt,
            "w1": np.ascontiguousarray(
                w1[e].reshape(KK1, P, MT1, P).transpose(1, 2, 0, 3).reshape(
                    P, MT1, KK1 * P)).astype(ml_dtypes.bfloat16),
            "w2": np.ascontiguousarray(
                w2[e].reshape(KK2, P, H).transpose(1, 0, 2)).astype(
                    ml_dtypes.bfloat16),
            "b1": np.ascontiguousarray(b1[e].reshape(MT1, P).T),
            "rw": np.ascontiguousarray(rwe.reshape(nmc, P).T),
        })

    res = run_bass_kernel_spmd(nc, in_maps, list(range(N_CORES)),
                               trace=_trace)
    last_result = res

    routed = np.zeros((npair, H), np.float32)
    for e in range(E):
        idx = order[starts[e]:starts[e + 1]]
        cnt = len(idx)
        ye = np.asarray(res.results[e]["y"]).astype(np.float32).reshape(C, H)
        routed[idx] = ye[:cnt]
        if b2[e].any():
            # b2 is applied on the host (exact fp32): y += rw * b2[expert]
            routed[idx] += fw[idx][:, None] * b2[e][None, :]

    y = routed.reshape(T, TOPK, H).sum(axis=1)
    return y.reshape(b, s, h).astype(np.float32)


# revision 9
# speedup vs baseline: 1.2164x; 1.2164x over previous
"""Grouped-GEMM MoE kernel for Trainium2, expert-parallel across 8 NeuronCores.

Strategy (matches the module's expert-parallel path):
  - Host routes: sort the T*topk routed pairs by expert id; core e gets the
    tokens routed to expert e (padded to a common capacity C).
  - Device (per core): Y = gelu(X @ W1 + b1) @ W2 + b2, scaled per-row by the
    routing weight. Two chained GEMMs on the PE array in bf16 with fp32 PSUM
    accumulation; gelu fused into the PSUM->SBUF eviction on the ACT engine.
  - Host combines: scatter rows back by pair index and sum the topk=2 slots.

Perf notes (trace-driven):
  - W1 is loaded as 16 independent 256 KB chunks (one per output m-tile) so
    the first matmul starts ~4us into the kernel instead of waiting ~34us for
    one monolithic 4 MB DMA.
  - Capacity is padded to a multiple of 128 (not 512): full 512-token tiles
    plus one remainder tile, cutting ~8% of padded matmul work.
  - Outputs are stored as bf16, halving the output DMA traffic.

Problem shapes (hardcoded per contract): B=4, S=4096, H=1024, F=2048, E=8,
TOPK=2.
"""

import sys

for _p in ("/opt/trn_rl_repo", "/opt/pypackages"):
    if _p not in sys.path:
        sys.path.insert(0, _p)

import ml_dtypes
import numpy as np

import concourse.bass as bass  # noqa: F401  (engine types come via bacc)
import concourse.mybir as mybir
import concourse.tile as tile
from concourse import bacc
from concourse.bass_utils import run_bass_kernel_spmd

H = 1024
F = 2048
E = 8
TOPK = 2
N_CORES = 8
P = 128
NTILE = 512          # full token-tile width (matmul moving dim)
KK1 = H // P         # 8  k-steps in GEMM1
KK2 = F // P         # 16 k-steps in GEMM2
MT1 = F // P         # 16 output m-tiles in GEMM1
NT2 = H // NTILE     # 2  output n-tiles in GEMM2
W2G = 4              # w2 DMA split: 4 chunks of 4 k-slices

BF16 = mybir.dt.bfloat16
F32 = mybir.dt.float32

_CACHE = {}
last_result = None   # BassKernelResults of the most recent device run


def _build(C):
    """Build + compile the per-core program for capacity C (multiple of 128)."""
    assert C % P == 0
    nfull = C // NTILE            # full 512-token tiles
    rem = C - nfull * NTILE       # remainder tile width (0/128/256/384)
    widths = [NTILE] * nfull + ([rem] if rem else [])
    nmc = C // P                  # token m-chunks (GEMM2 output rows)

    nc = bacc.Bacc("TRN2", target_bir_lowering=False, debug=False,
                   num_devices=N_CORES)

    # DRAM I/O. Layouts are chosen so every DMA is a plain strided copy:
    #   xt[p, kk, c]       = X[c, kk*128+p]   (tokens transposed, H in 8x128)
    #   w1[p, m, kk*128+j] = W1[kk*128+p, m*128+j]  (per-m contiguous chunks)
    #   w2[p, kk, j]       = W2[kk*128+p, j]
    #   b1[p, m]           = b1[m*128+p]
    #   b2r[p, j]          = b2[j]            (pre-replicated across partitions)
    #   rw[p, mc]          = rweight[mc*128+p]
    #   y[mc, p, j]        = Y[mc*128+p, j]   (bf16)
    xt_d = nc.dram_tensor("xt", [P, KK1, C], BF16, kind="ExternalInput")
    w1_d = nc.dram_tensor("w1", [P, MT1, KK1 * P], BF16, kind="ExternalInput")
    w2_d = nc.dram_tensor("w2", [P, KK2, H], BF16, kind="ExternalInput")
    b1_d = nc.dram_tensor("b1", [P, MT1], F32, kind="ExternalInput")
    rw_d = nc.dram_tensor("rw", [P, nmc], F32, kind="ExternalInput")
    y_d = nc.dram_tensor("y", [nmc, P, H], BF16, kind="ExternalOutput")

    with tile.TileContext(nc) as tc:
        with (
            tc.tile_pool(name="const", bufs=1) as const,
            tc.tile_pool(name="xin", bufs=3) as xin,
            tc.tile_pool(name="gact", bufs=3) as gact,
            tc.tile_pool(name="yout", bufs=4) as yout,
            tc.tile_pool(name="psg", bufs=4, space="PSUM") as psg,
            tc.tile_pool(name="psy", bufs=3, space="PSUM") as psy,
        ):
            # First x tile in 4 chunks (2 k-slices each) so GEMM1 m=0 can
            # start as soon as the first 256 KB lands. DMA issue on the Sync
            # engine serializes at ~0.6us per dma_start, so the front only
            # carries the chunks that actually gate compute; the bulk of the
            # weights goes out as a few large transfers.
            xt0 = xin.tile([P, KK1, NTILE if widths else P], BF16, tag="xt")
            for q in range(4):
                nc.sync.dma_start(xt0[:, 2 * q:2 * q + 2, :],
                                  xt_d[:, 2 * q:2 * q + 2, :widths[0]])

            # W1 m-tiles 0-3 as independent 256 KB chunks issued from the
            # Scalar engine (its HWDGE queues are separate from Sync's, so
            # these land in parallel with the xt0 chunks).
            w1_t = []
            for m in range(4):
                t = const.tile([P, KK1 * P], BF16, tag=f"w1_{m}")
                nc.scalar.dma_start(t[:], w1_d[:, m, :])
                w1_t.append(t)
            b1_sb = const.tile([P, MT1], F32)
            nc.scalar.dma_start(b1_sb[:], b1_d[:])

            # Rest of W1 in two 1.5 MB transfers (needed ~20us+ in).
            w1g = []
            for g in range(2):
                t = const.tile([P, 6, KK1 * P], BF16, tag=f"w1g_{g}")
                nc.sync.dma_start(t[:], w1_d[:, 4 + 6 * g:10 + 6 * g, :])
                w1g.append(t)

            def w1s(m, kk):
                if m < 4:
                    return w1_t[m][:, kk * P:(kk + 1) * P]
                g, mi = divmod(m - 4, 6)
                return w1g[g][:, mi, kk * P:(kk + 1) * P]

            # W2 as one 4 MB transfer (needed only when GEMM2 starts ~50us).
            w2_sb = const.tile([P, KK2, H], BF16)
            nc.sync.dma_start(w2_sb[:], w2_d[:])

            rw_sb = const.tile([P, nmc], F32)
            nc.sync.dma_start(rw_sb[:], rw_d[:])

            cbase = 0
            for ct, w in enumerate(widths):
                if ct == 0:
                    xt_sb = xt0
                else:
                    xt_sb = xin.tile([P, KK1, w], BF16, tag="xt")
                    nc.sync.dma_start(
                        xt_sb[:], xt_d[:, :, cbase:cbase + w])

                # GEMM1: GT[f, c] = sum_h W1[h, f] * XT[h, c], then
                # gelu(.+b1) on eviction. F on partitions, tokens on free.
                gt_sb = gact.tile([P, MT1, w], BF16, tag="gt")
                for m in range(MT1):
                    pg = psg.tile([P, w], F32, tag="pg")
                    for kk in range(KK1):
                        nc.tensor.matmul(
                            pg[:],
                            w1s(m, kk),
                            xt_sb[:, kk, :w],
                            start=(kk == 0), stop=(kk == KK1 - 1))
                    nc.scalar.activation(
                        gt_sb[:, m, :], pg[:],
                        mybir.ActivationFunctionType.Gelu,
                        bias=b1_sb[:, m:m + 1])

                # GEMM2: Y[c, j] = sum_f GT[f, c] * W2[f, j]; tokens on
                # partitions. Evict: (+b2) * routing_weight -> bf16, store.
                for mo in range(w // P):
                    mc = cbase // P + mo
                    for n in range(NT2):
                        py = psy.tile([P, NTILE], F32, tag="py")
                        for kk in range(KK2):
                            nc.tensor.matmul(
                                py[:],
                                gt_sb[:, kk, mo * P:(mo + 1) * P],
                                w2_sb[:, kk, n * NTILE:(n + 1) * NTILE],
                                start=(kk == 0), stop=(kk == KK2 - 1))
                        yo = yout.tile([P, NTILE], BF16, tag="yo")
                        nc.vector.tensor_scalar_mul(
                            yo[:], py[:], rw_sb[:, mc:mc + 1])
                        nc.sync.dma_start(
                            y_d[mc, :, n * NTILE:(n + 1) * NTILE], yo[:])
                cbase += w

    nc.compile()
    return nc


def kernel(hidden_states, expert_weights, top_experts, w1, b1, w2, b2,
           _trace=False):
    global last_result
    x = np.asarray(hidden_states, dtype=np.float32)
    fw = np.asarray(expert_weights, dtype=np.float32).reshape(-1)
    fe = np.asarray(top_experts).reshape(-1).astype(np.int64)
    w1 = np.asarray(w1, dtype=np.float32)
    b1 = np.asarray(b1, dtype=np.float32)
    w2 = np.asarray(w2, dtype=np.float32)
    b2 = np.asarray(b2, dtype=np.float32)

    b, s, h = x.shape
    T = b * s
    xf = x.reshape(T, h)
    npair = T * TOPK

    # Host-side routing: stable sort of pair indices by expert id.
    order = np.argsort(fe, kind="stable")
    counts = np.bincount(fe, minlength=E)
    starts = np.concatenate([[0], np.cumsum(counts)])
    C = max(int(-(-counts.max() // P)) * P, P)
    nmc = C // P

    key = C
    if key not in _CACHE:
        _CACHE[key] = _build(C)
    nc = _CACHE[key]

    in_maps = []
    for e in range(E):
        idx = order[starts[e]:starts[e + 1]]
        cnt = len(idx)
        xe = np.zeros((C, H), np.float32)
        xe[:cnt] = xf[idx // TOPK]
        xt = np.ascontiguousarray(
            xe.T.reshape(KK1, P, C).transpose(1, 0, 2)).astype(
                ml_dtypes.bfloat16)
        rwe = np.zeros(C, np.float32)
        rwe[:cnt] = fw[idx]
        in_maps.append({
            "xt": xt,
            "w1": np.ascontiguousarray(
                w1[e].reshape(KK1, P, MT1, P).transpose(1, 2, 0, 3).reshape(
                    P, MT1, KK1 * P)).astype(ml_dtypes.bfloat16),
            "w2": np.ascontiguousarray(
                w2[e].reshape(KK2, P, H).transpose(1, 0, 2)).astype(
                    ml_dtypes.bfloat16),
            "b1": np.ascontiguousarray(b1[e].reshape(MT1, P).T),
            "rw": np.ascontiguousarray(rwe.reshape(nmc, P).T),
        })

    res = run_bass_kernel_spmd(nc, in_maps, list(range(N_CORES)),
                               trace=_trace)
    last_result = res

    routed = np.zeros((npair, H), np.float32)
    for e in range(E):
        idx = order[starts[e]:starts[e + 1]]
        cnt = len(idx)
        ye = np.asarray(res.results[e]["y"]).astype(np.float32).reshape(C, H)
        routed[idx] = ye[:cnt]
        if b2[e].any():
            # b2 is applied on the host (exact fp32): y += rw * b2[expert]
            routed[idx] += fw[idx][:, None] * b2[e][None, :]

    y = routed.reshape(T, TOPK, H).sum(axis=1)
    return y.reshape(b, s, h).astype(np.float32)


# revision 10
# speedup vs baseline: 1.2172x; 1.0006x over previous
"""Grouped-GEMM MoE kernel for Trainium2, expert-parallel across 8 NeuronCores.

Strategy (matches the module's expert-parallel path):
  - Host routes: sort the T*topk routed pairs by expert id; core e gets the
    tokens routed to expert e (padded to a common capacity C).
  - Device (per core): Y = gelu(X @ W1 + b1) @ W2 + b2, scaled per-row by the
    routing weight. Two chained GEMMs on the PE array in bf16 with fp32 PSUM
    accumulation; gelu fused into the PSUM->SBUF eviction on the ACT engine.
  - Host combines: scatter rows back by pair index and sum the topk=2 slots.

Perf notes (trace-driven):
  - W1 is loaded as 16 independent 256 KB chunks (one per output m-tile) so
    the first matmul starts ~4us into the kernel instead of waiting ~34us for
    one monolithic 4 MB DMA.
  - Capacity is padded to a multiple of 128 (not 512): full 512-token tiles
    plus one remainder tile, cutting ~8% of padded matmul work.
  - Outputs are stored as bf16, halving the output DMA traffic.

Problem shapes (hardcoded per contract): B=4, S=4096, H=1024, F=2048, E=8,
TOPK=2.
"""

import sys

for _p in ("/opt/trn_rl_repo", "/opt/pypackages"):
    if _p not in sys.path:
        sys.path.insert(0, _p)

import ml_dtypes
import numpy as np

import concourse.bass as bass  # noqa: F401  (engine types come via bacc)
import concourse.mybir as mybir
import concourse.tile as tile
from concourse import bacc
from concourse.bass_utils import run_bass_kernel_spmd

H = 1024
F = 2048
E = 8
TOPK = 2
N_CORES = 8
P = 128
NTILE = 512          # full token-tile width (matmul moving dim)
KK1 = H // P         # 8  k-steps in GEMM1
KK2 = F // P         # 16 k-steps in GEMM2
MT1 = F // P         # 16 output m-tiles in GEMM1
NT2 = H // NTILE     # 2  output n-tiles in GEMM2
W2G = 4              # w2 DMA split: 4 chunks of 4 k-slices

BF16 = mybir.dt.bfloat16
F32 = mybir.dt.float32

_CACHE = {}
last_result = None   # BassKernelResults of the most recent device run


def _build(C):
    """Build + compile the per-core program for capacity C (multiple of 128)."""
    assert C % P == 0
    nfull = C // NTILE            # full 512-token tiles
    rem = C - nfull * NTILE       # remainder width (0/128/256/384)
    # The remainder rides along with the last full tile: each GEMM1 k-step
    # loads the stationary weight once and streams 512+rem tokens through it
    # as two chained matmuls into separate PSUM banks. This avoids a separate
    # LDWEIGHTS-bound remainder pass.
    if nfull == 0:
        widths = [rem]
    elif rem:
        widths = [NTILE] * (nfull - 1) + [NTILE + rem]
    else:
        widths = [NTILE] * nfull
    nmc = C // P                  # token m-chunks (GEMM2 output rows)

    nc = bacc.Bacc("TRN2", target_bir_lowering=False, debug=False,
                   num_devices=N_CORES)

    # DRAM I/O. Layouts are chosen so every DMA is a plain strided copy:
    #   xt[p, kk, c]       = X[c, kk*128+p]   (tokens transposed, H in 8x128)
    #   w1[p, m, kk*128+j] = W1[kk*128+p, m*128+j]  (per-m contiguous chunks)
    #   w2[p, kk, j]       = W2[kk*128+p, j]
    #   b1[p, m]           = b1[m*128+p]
    #   b2r[p, j]          = b2[j]            (pre-replicated across partitions)
    #   rw[p, mc]          = rweight[mc*128+p]
    #   y[mc, p, j]        = Y[mc*128+p, j]   (bf16)
    xt_d = nc.dram_tensor("xt", [P, KK1, C], BF16, kind="ExternalInput")
    w1_d = nc.dram_tensor("w1", [P, MT1, KK1 * P], BF16, kind="ExternalInput")
    w2_d = nc.dram_tensor("w2", [P, KK2, H], BF16, kind="ExternalInput")
    b1_d = nc.dram_tensor("b1", [P, MT1], F32, kind="ExternalInput")
    rw_d = nc.dram_tensor("rw", [P, nmc], F32, kind="ExternalInput")
    y_d = nc.dram_tensor("y", [nmc, P, H], BF16, kind="ExternalOutput")

    with tile.TileContext(nc) as tc:
        with (
            tc.tile_pool(name="const", bufs=1) as const,
            tc.tile_pool(name="xin", bufs=3) as xin,
            tc.tile_pool(name="gact", bufs=3) as gact,
            tc.tile_pool(name="yout", bufs=4) as yout,
            tc.tile_pool(name="psg", bufs=3, space="PSUM") as psg,
            tc.tile_pool(name="psgr", bufs=2, space="PSUM") as psgr,
            tc.tile_pool(name="psy", bufs=3, space="PSUM") as psy,
        ):
            # First x tile in 4 chunks (2 k-slices each) so GEMM1 m=0 can
            # start as soon as the first 256 KB lands. DMA issue on the Sync
            # engine serializes at ~0.6us per dma_start, so the front only
            # carries the chunks that actually gate compute; the bulk of the
            # weights goes out as a few large transfers.
            xt0 = xin.tile([P, KK1, NTILE if widths else P], BF16, tag="xt")
            for q in range(4):
                nc.sync.dma_start(xt0[:, 2 * q:2 * q + 2, :],
                                  xt_d[:, 2 * q:2 * q + 2, :widths[0]])

            # W1 m-tiles 0-3 as independent 256 KB chunks issued from the
            # Scalar engine (its HWDGE queues are separate from Sync's, so
            # these land in parallel with the xt0 chunks).
            w1_t = []
            for m in range(4):
                t = const.tile([P, KK1 * P], BF16, tag=f"w1_{m}")
                nc.scalar.dma_start(t[:], w1_d[:, m, :])
                w1_t.append(t)
            b1_sb = const.tile([P, MT1], F32)
            nc.scalar.dma_start(b1_sb[:], b1_d[:])

            # Rest of W1 in two 1.5 MB transfers (needed ~20us+ in).
            w1g = []
            for g in range(2):
                t = const.tile([P, 6, KK1 * P], BF16, tag=f"w1g_{g}")
                nc.sync.dma_start(t[:], w1_d[:, 4 + 6 * g:10 + 6 * g, :])
                w1g.append(t)

            def w1s(m, kk):
                if m < 4:
                    return w1_t[m][:, kk * P:(kk + 1) * P]
                g, mi = divmod(m - 4, 6)
                return w1g[g][:, mi, kk * P:(kk + 1) * P]

            # W2 as one 4 MB transfer (needed only when GEMM2 starts ~50us).
            w2_sb = const.tile([P, KK2, H], BF16)
            nc.sync.dma_start(w2_sb[:], w2_d[:])

            rw_sb = const.tile([P, nmc], F32)
            nc.sync.dma_start(rw_sb[:], rw_d[:])

            cbase = 0
            for ct, w in enumerate(widths):
                if ct == 0:
                    xt_sb = xt0
                else:
                    xt_sb = xin.tile([P, KK1, w], BF16, tag="xt")
                    nc.sync.dma_start(
                        xt_sb[:], xt_d[:, :, cbase:cbase + w])

                # GEMM1: GT[f, c] = sum_h W1[h, f] * XT[h, c], then
                # gelu(.+b1) on eviction. F on partitions, tokens on free.
                wf = min(w, NTILE)       # main chain width
                wr = w - wf              # rider chain width (last tile only)
                gt_sb = gact.tile([P, MT1, w], BF16, tag="gt")
                for m in range(MT1):
                    pg = psg.tile([P, wf], F32, tag="pg")
                    if wr:
                        pgr = psgr.tile([P, wr], F32, tag="pgr")
                    for kk in range(KK1):
                        nc.tensor.matmul(
                            pg[:],
                            w1s(m, kk),
                            xt_sb[:, kk, :wf],
                            start=(kk == 0), stop=(kk == KK1 - 1))
                        if wr:
                            nc.tensor.matmul(
                                pgr[:],
                                w1s(m, kk),
                                xt_sb[:, kk, wf:w],
                                start=(kk == 0), stop=(kk == KK1 - 1))
                    nc.scalar.activation(
                        gt_sb[:, m, :wf], pg[:],
                        mybir.ActivationFunctionType.Gelu,
                        bias=b1_sb[:, m:m + 1])
                    if wr:
                        nc.scalar.activation(
                            gt_sb[:, m, wf:w], pgr[:],
                            mybir.ActivationFunctionType.Gelu,
                            bias=b1_sb[:, m:m + 1])

                # GEMM2: Y[c, j] = sum_f GT[f, c] * W2[f, j]; tokens on
                # partitions. Evict: (+b2) * routing_weight -> bf16, store.
                for mo in range(w // P):
                    mc = cbase // P + mo
                    for n in range(NT2):
                        py = psy.tile([P, NTILE], F32, tag="py")
                        for kk in range(KK2):
                            nc.tensor.matmul(
                                py[:],
                                gt_sb[:, kk, mo * P:(mo + 1) * P],
                                w2_sb[:, kk, n * NTILE:(n + 1) * NTILE],
                                start=(kk == 0), stop=(kk == KK2 - 1))
                        yo = yout.tile([P, NTILE], BF16, tag="yo")
                        nc.vector.tensor_scalar_mul(
                            yo[:], py[:], rw_sb[:, mc:mc + 1])
                        nc.sync.dma_start(
                            y_d[mc, :, n * NTILE:(n + 1) * NTILE], yo[:])
                cbase += w

    nc.compile()
    return nc


def kernel(hidden_states, expert_weights, top_experts, w1, b1, w2, b2,
           _trace=False):
    global last_result
    x = np.asarray(hidden_states, dtype=np.float32)
    fw = np.asarray(expert_weights, dtype=np.float32).reshape(-1)
    fe = np.asarray(top_experts).reshape(-1).astype(np.int64)
    w1 = np.asarray(w1, dtype=np.float32)
    b1 = np.asarray(b1, dtype=np.float32)
    w2 = np.asarray(w2, dtype=np.float32)
    b2 = np.asarray(b2, dtype=np.float32)

    b, s, h = x.shape
    T = b * s
    xf = x.reshape(T, h)
    npair = T * TOPK

    # Host-side routing: stable sort of pair indices by expert id.
    order = np.argsort(fe, kind="stable")
    counts = np.bincount(fe, minlength=E)
    starts = np.concatenate([[0], np.cumsum(counts)])
    C = max(int(-(-counts.max() // P)) * P, P)
    nmc = C // P

    key = C
    if key not in _CACHE:
        _CACHE[key] = _build(C)
    nc = _CACHE[key]

    in_maps = []
    for e in range(E):
        idx = order[starts[e]:starts[e + 1]]
        cnt = len(idx)
        xe = np.zeros((C, H), np.float32)
        xe[:cnt] = xf[idx // TOPK]
        xt = np.ascontiguousarray(
            xe.T.reshape(KK1, P, C).transpose(1, 0, 2)).astype(
                ml_dtypes.bfloat16)
        rwe = np.zeros(C, np.float32)
        rwe[:cnt] = fw[idx]
        in_maps.append({
            "xt": xt,
            "w1": np.ascontiguousarray(
                w1[e].reshape(KK1, P, MT1, P).transpose(1, 2, 0, 3).reshape(
                    P, MT1, KK1 * P)).astype(ml_dtypes.bfloat16),
            "w2": np.ascontiguousarray(
                w2[e].reshape(KK2, P, H).transpose(1, 0, 2)).astype(
                    ml_dtypes.bfloat16),
            "b1": np.ascontiguousarray(b1[e].reshape(MT1, P).T),
            "rw": np.ascontiguousarray(rwe.reshape(nmc, P).T),
        })

    res = run_bass_kernel_spmd(nc, in_maps, list(range(N_CORES)),
                               trace=_trace)
    last_result = res

    routed = np.zeros((npair, H), np.float32)
    for e in range(E):
        idx = order[starts[e]:starts[e + 1]]
        cnt = len(idx)
        ye = np.asarray(res.results[e]["y"]).astype(np.float32).reshape(C, H)
        routed[idx] = ye[:cnt]
        if b2[e].any():
            # b2 is applied on the host (exact fp32): y += rw * b2[expert]
            routed[idx] += fw[idx][:, None] * b2[e][None, :]

    y = routed.reshape(T, TOPK, H).sum(axis=1)
    return y.reshape(b, s, h).astype(np.float32)
